# revision 18
# baseline (speedup 1.0000x reference)
"""Multi-head self-attention (B=2, C=512, H=W=64, 8 heads) on 8 Trainium2 cores.

Sharding: data-parallel over B x head-parallel (2 heads/core). Core c handles
batch b = c//4 and heads {2*(c%4), 2*(c%4)+1} -- a contiguous 128-wide slice of
the 512-dim channel space.

Structure (v4 -- row-tiled concurrent score matmuls):
  - x[b] viewed as [C, S] is tok^T already (S = H*W = 4096 tokens); x and all
    weights are shipped as bf16.
  - q^T computed as [d2=128, S] bf16 with bias; k^T stored UNPADDED as
    kT_both[128, S]: rows 0-63 = head0 dims, 64-127 = head1 dims (k bias
    dropped: its score terms are query-only and cancel in softmax).
  - scores: per unit (sb, t) = (512-query strip, 128-key chunk), TWO
    CONCURRENT row-tiled K=64 matmuls (head0 in PE row-group 0-1, head1 in
    row-group 2-3, via base_partition 0/64) write pssc[128, 2(head), 512]
    in two PSUM banks. 2x the old zero-padded K=128 formulation: the PE
    streams 64+64 moving rows while draining 2x128 outputs into 2 banks.
  - exp: scores/8 ~ N(0, 0.33) so no max-subtraction. Split across engines:
      * ACT: exp activation (scale=1/8) writing float8e4.
      * DVE: Schraudolph-style magic exp -- uint8 = trunc(score*log2e +
        56.15) bitcast as float8e4 is exp(score/8) * 2^0.019 with ~4% rms
        error (the uniform factor cancels in softmax). One tensor_scalar op.
    exp output lands in expP[128, 2(head), 2(chunk parity), 512] so that a
    pair of units (t=2i, 2i+1) forms the [Ki, 2, N] moving operand per head.
  - attn.V: fp8e4 DoubleRow matmuls (two 128-key chunks per pass, K=256;
    the DR matmul streams its 1024 fp8 moving columns at 2 B/cycle/lane =
    216 ns, measured): stationary v8[128, 2, 96] = [v(64) | ones | zero pad]
    (ones column yields the softmax denominator in psav row 64), moving
    expP[:, h, :, :] fp8, out psav[96, 512] f32, one per (sb, head).
  - normalization: the [1,512] denominator row is DMA-spread to [128,4]
    so the DVE reciprocal runs wide, then DMA'd back, gpsimd-broadcast,
    and multiplied into outT2. Emitted in 3 phases several units apart so
    the in-order DVE never head-of-line blocks on an in-flight DMA; AV
    numerators are copied off PSUM in phase 1 so the psav bank frees early.
  - projection: input-column sharded; proj matmuls borrow score-pool PSUM
    tiles; PSUM->SBUF copies run on ACT; bias terms (bp and bv@Wp^T) are
    added on the host during the partial-sum gather.
  - pipeline: scores/exp stream runs AV_LAG units ahead of the AV consumer;
    the first 24 units are interleaved into the QKV prologue.

Error: fp8 quantization of exp/V ~0.7%, magic-exp ~0.3%, bf16 ~0.1% =>
measured 8.4e-3 against the 2e-2 gate (norm-relative, diluted by the
bias-dominated output norm).
"""

import os
import sys

sys.path.insert(0, "/opt/trn_rl_repo")

import numpy as np

NCORES = 8
B, C, HH, WW = 2, 512, 64, 64
S = HH * WW            # 4096 tokens
NH, D = 8, 64          # heads, head dim
DSL = 128              # per-core d-slice (2 heads)
CC = C // 128          # 4 contraction chunks
TCH = S // 128         # 32 key chunks
T2 = TCH // 2          # 16 key chunk-pairs
SBLK = 512             # queries per attention strip
NSB = S // SBLK        # 8

LOG2E = float(1.4426950408889634)
MAGIC_C = float(os.environ.get("MAGIC_C", "56.15"))
# t indices (0..31 within each sb group) whose exp runs on DVE via magic trick
DVE_T = frozenset(
    int(t) for t in os.environ.get(
        "DVE_T",
        "1,3,5,7,9,11,13,15,17,19,21,23,25,27,29",
    ).split(",") if t != ""
)
AV_LAG = int(os.environ.get("AV_LAG", "4"))


_cached = {}

LAST_EXEC_NS = None
LAST_RESULTS = None


def _build():
    import concourse.mybir as mybir
    import concourse.tile as tile
    from bass_rust import add_dep_helper
    from concourse import bacc
    from concourse.masks import make_identity

    f32 = mybir.dt.float32
    f32r = mybir.dt.float32r
    bf16 = mybir.dt.bfloat16
    f8 = mybir.dt.float8e4
    u8 = mybir.dt.uint8
    AF = mybir.ActivationFunctionType
    DR = mybir.MatmulPerfMode.DoubleRow

    nc = bacc.Bacc("TRN2", target_bir_lowering=False, debug=False,
                   num_devices=NCORES)

    xb = nc.dram_tensor("xb", [C, S], bf16, kind="ExternalInput")
    wq = nc.dram_tensor("wq", [128, CC, 128], bf16, kind="ExternalInput")
    wk = nc.dram_tensor("wk", [128, CC, 128], bf16, kind="ExternalInput")
    wv = nc.dram_tensor("wv", [128, CC, 128], bf16, kind="ExternalInput")
    wp = nc.dram_tensor("wp", [128, CC, 128], bf16, kind="ExternalInput")
    bq = nc.dram_tensor("bq", [128, 1], f32, kind="ExternalInput")
    o = nc.dram_tensor("o", [C, S], bf16, kind="ExternalOutput")

    with tile.TileContext(nc) as tc:
        with (
            tc.tile_pool(name="weights", bufs=1) as wpool,
            tc.tile_pool(name="tok", bufs=1) as tokpool,
            tc.tile_pool(name="qkv", bufs=1) as qkvpool,
            tc.tile_pool(name="exps", bufs=6) as exppool,
            tc.tile_pool(name="norm", bufs=10) as normpool,
            tc.tile_pool(name="outp", bufs=5) as outpool,
        ):
            wq_sb = wpool.tile([128, CC, 128], bf16, name="wq_sb")
            nc.sync.dma_start(out=wq_sb[:], in_=wq.ap())
            wk_sb = wpool.tile([128, CC, 128], bf16, name="wk_sb")
            nc.sync.dma_start(out=wk_sb[:], in_=wk.ap())
            wv_sb = wpool.tile([128, CC, 128], bf16, name="wv_sb")
            nc.sync.dma_start(out=wv_sb[:], in_=wv.ap())
            wp_sb = wpool.tile([128, CC, 128], bf16, name="wp_sb")
            nc.sync.dma_start(out=wp_sb[:], in_=wp.ap())
            bq_sb = wpool.tile([128, 1], f32, name="bq_sb")
            nc.sync.dma_start(out=bq_sb[:], in_=bq.ap())

            # tok^T in [partition, c_chunk, s] layout; DMA rearranges rows.
            tok_sb = tokpool.tile([128, CC, S], bf16, name="tok_sb")
            x_re = xb.ap().rearrange("(cc p) s -> p cc s", p=128)
            for qtr in range(4):
                for hf in range(2):
                    for cc in range(CC):
                        a = qtr * (S // 4) + hf * (S // 8)
                        sl = slice(a, a + S // 8)
                        nc.sync.dma_start(out=tok_sb[:, cc, sl],
                                          in_=x_re[:, cc, sl])

            # q/k in bf16. kT_both rows 0-63 = head0 dims, 64-127 = head1.
            qT2 = qkvpool.tile([128, S], bf16, name="qT2")
            kT_both = qkvpool.tile([128, S], bf16, name="kT_both")
            # v in fp8, chunk-pair layout for DoubleRow: [t, t2, j, 96] with
            # a fused ones column (65th) per chunk for the denominator.
            # DoubleRow stationary free (2*MV) must be a multiple of 64.
            MV = 96
            v8_0 = qkvpool.tile([128, T2, 2, MV], f8, name="v8_0")
            v8_1 = qkvpool.tile([128, T2, 2, MV], f8, name="v8_1")
            ones32 = qkvpool.tile([128, T2, 2], f32, name="ones32")
            nc.vector.memset(ones32[:], 1.0)
            nc.vector.tensor_copy(v8_0[:, :, :, 64], ones32[:])
            nc.vector.tensor_copy(v8_1[:, :, :, 64], ones32[:])
            zpad = qkvpool.tile([128, T2, 2, MV - 65], f32, name="zpad")
            nc.vector.memset(zpad[:], 0.0)
            nc.vector.tensor_copy(v8_0[:, :, :, 65:MV], zpad[:])
            nc.vector.tensor_copy(v8_1[:, :, :, 65:MV], zpad[:])

            outT2 = qkvpool.tile([128, S], bf16, name="outT2")

            ident = qkvpool.tile([128, 128], f32, name="ident")
            make_identity(nc, ident[:])

            ctx_psav = tc.tile_pool(name="psav", bufs=2, space="PSUM")
            pavp = ctx_psav.__enter__()
            psavs = {}
            exp_state = {"emitted": 0, "av_done": 0, "pending": []}

            # ---- unit schedule ------------------------------------------
            # A unit is (sb, t): one 512-query strip, one 128-key chunk,
            # both heads (concurrent row-tiled matmuls). AV consumes units
            # in chunk-parity pairs per head, accumulating over t2 = t//2
            # within each (sb, head) group. The first 24 units front-load
            # into the QKV prologue (chunk availability grows with token
            # quarters), at most 2 (sb, head) AV groups in flight.
            units = [(sb, t) for sb in range(NSB) for t in range(TCH)]
            N_EARLY_SC = [8, 16, 24]
            N_EARLY_AV = [6, 14, 22]   # AV emissions = (pair, head) steps

            def is_dve_unit(i):
                if i < 20:
                    return False  # prologue units: DVE is busy with QKV
                return units[i][1] in DVE_T

            def emit_exp(pssc_ap, out_ap, dve):
                # out_ap: [128, 2, 512] u8 view (head-major, parity slice)
                if dve:
                    nc.vector.tensor_scalar(
                        out_ap, pssc_ap, LOG2E, MAGIC_C,
                        mybir.AluOpType.mult, mybir.AluOpType.add,
                    )
                else:
                    nc.scalar.activation(out_ap.bitcast(f8), pssc_ap,
                                         AF.Exp, scale=0.125)

            def emit_scores(u, pool):
                sb, t = u
                s0 = sb * SBLK
                t0 = t * 128
                pssc = pool.tile([128, 2, SBLK], f32, name="pssc")
                for h in range(2):
                    nc.tensor.matmul(
                        pssc[:, h, :],
                        kT_both[h * 64:(h + 1) * 64, t0:t0 + 128],
                        qT2[h * 64:(h + 1) * 64, s0:s0 + SBLK],
                        start=True, stop=True,
                    )
                return pssc

            # expP per unit-pair: [128, 2(parity), 2(head), 512] -- parity
            # outer so each unit's exp writes one contiguous [128, 1024]
            # range (strided ACT output costs ~12%).
            def exp_dest(u, expP):
                return expP[:, u[1] % 2, :, :]

            def emit_av(u, expP, h):
                # u = the ODD unit of the pair; expP holds both parities
                sb, t = u
                t2 = t // 2
                v8 = v8_0 if h == 0 else v8_1
                if (sb, h) not in psavs:
                    psavs[(sb, h)] = pavp.tile([MV, SBLK], f32, name="psav")
                psav = psavs[(sb, h)]
                return nc.tensor.matmul(
                    psav[:],
                    v8[:, t2, :, :],
                    expP[:, :, h, :].bitcast(f8),
                    start=(t2 == 0), stop=(t2 == T2 - 1),
                    perf_mode=DR,
                )

            # The normalization chain (denominator reciprocal + broadcast +
            # multiply) crosses engines five times; spread the [1, 512]
            # denominator row across 128 partitions with a pair of tiny
            # SBUF<->SBUF DMAs and run the reciprocal at [128, 4]. The three
            # DVE steps are emitted a few units apart (phases) so the
            # in-order DVE never head-of-line blocks on an in-flight DMA.
            def norm_ph1(st):
                # one copy moves numerators AND the denominator row (65)
                # off PSUM; the spread-DMA then reads the SBUF copy.
                st["avs"] = normpool.tile([65, SBLK], f32, name="avs")
                nc.vector.tensor_copy(st["avs"][:], st["psav"][0:65, :])
                st["dsp"] = normpool.tile([128, SBLK // 128], f32, name="dsp")
                nc.sync.dma_start(out=st["dsp"][:], in_=st["avs"][64:65, :])

            def norm_ph2(st):
                st["rsp"] = normpool.tile([128, SBLK // 128], f32, name="rsp")
                nc.vector.reciprocal(st["rsp"][:], st["dsp"][:])
                st["recip"] = normpool.tile([1, SBLK], f32, name="recip")
                nc.sync.dma_start(out=st["recip"][:], in_=st["rsp"][:])
                st["rb"] = normpool.tile([64, SBLK], f32, name="rb")
                nc.gpsimd.partition_broadcast(st["rb"][:], st["recip"][:])

            def norm_ph3(st):
                # on gpsimd: SBUF-only multiply, frees the DVE for exp
                sb, h = st["sb"], st["h"]
                p0 = h * 64
                nc.gpsimd.tensor_mul(
                    outT2[p0:p0 + 64, sb * SBLK:(sb + 1) * SBLK],
                    st["avs"][0:64, :], st["rb"][:],
                )

            NORM_PHASES = (norm_ph1, norm_ph2, norm_ph3)

            # ---- fused Q/K/V prologue, quarter-major so compute chases
            # the x DMA. V is computed transposed (efficient N=512 matmuls)
            # and flipped into [t, d] fp8 layout with PE transposes.
            with (
                tc.tile_pool(name="psqk", bufs=2, space="PSUM") as pqkp,
                tc.tile_pool(name="pssce", bufs=2, space="PSUM") as pscep,
                tc.tile_pool(name="vt", bufs=2) as vtpool,
            ):
                def early_unit(i):
                    # scores + exp for unit i (sb=0, t=i) in prologue PSUM
                    u = units[i]
                    pssc = emit_scores(u, pscep)
                    if u[1] % 2 == 0:
                        expP = exppool.tile([128, 2, 2, SBLK], u8,
                                            name="expP")
                        exp_state["curP"] = expP
                    else:
                        expP = exp_state["curP"]
                    # prologue exp always on ACT (DVE busy with QKV), split
                    # per head so each [128, 512] lands as one op
                    nc.scalar.activation(
                        exp_dest(u, expP).bitcast(f8), pssc[:],
                        AF.Exp, scale=0.125)
                    return expP

                def early_scores_advance(k):
                    st = exp_state
                    while st["emitted"] < k:
                        i = st["emitted"]
                        expP = early_unit(i)
                        if units[i][1] % 2 == 1:
                            st["pending"].append((units[i], expP))
                        st["emitted"] = i + 1

                def early_av_advance(k):
                    # k counts AV emissions (pair x head steps)
                    st = exp_state
                    while st["av_done"] < k and st["pending"]:
                        u, expP = st["pending"][0]
                        h = st.get("next_h", 0)
                        emit_av(u, expP, h)
                        st["av_done"] += 1
                        if h == 0:
                            st["next_h"] = 1
                        else:
                            st["next_h"] = 0
                            st["pending"].pop(0)

                for qtr in range(4):
                    for which in range(3):
                        w_sb = (wq_sb, wk_sb, wv_sb)[which]
                        for scq in range(2):
                            sc = qtr * 2 + scq
                            s0 = sc * 512
                            psqk = pqkp.tile([128, 512], f32, name="psqk")
                            for cc in range(CC):
                                nc.tensor.matmul(
                                    psqk[:],
                                    w_sb[:, cc, :],
                                    tok_sb[:, cc, s0:s0 + 512],
                                    start=(cc == 0), stop=(cc == CC - 1),
                                )
                            if which == 0:
                                nc.vector.tensor_scalar_add(
                                    qT2[:, s0:s0 + 512], psqk[:], bq_sb[:, 0:1]
                                )
                            elif which == 1:
                                # no k bias: its score terms are query-only
                                # and cancel in softmax
                                nc.vector.tensor_copy(
                                    kT_both[:, s0:s0 + 512], psqk[:]
                                )
                            else:
                                vt = vtpool.tile([128, 512], f32r, name="vt")
                                nc.vector.tensor_copy(vt[:], psqk[:])
                                # 4 transposes into one PSUM tile, then two
                                # strided fp8 copies peel the head halves.
                                pst4 = pqkp.tile([128, 512], f32, name="psqk")
                                for tt in range(4):
                                    nc.tensor.transpose(
                                        pst4[:, tt * 128:(tt + 1) * 128],
                                        vt[:, tt * 128:(tt + 1) * 128]
                                        .bitcast(f32),
                                        ident[:],
                                    )
                                # chunks sc*4 .. sc*4+3 -> t2 = sc*2, sc*2+1
                                t2a = sc * 2
                                src0 = pst4[:].rearrange(
                                    "p (c d) -> p c d", c=4)[:, :, 0:64]
                                src1 = pst4[:].rearrange(
                                    "p (c d) -> p c d", c=4)[:, :, 64:128]
                                dst0 = v8_0[:, t2a:t2a + 2, :, 0:64]
                                dst1 = v8_1[:, t2a:t2a + 2, :, 0:64]
                                nc.vector.tensor_copy(dst0, src0)
                                nc.vector.tensor_copy(dst1, src1)
                        if qtr < 3 and which == 1:
                            early_scores_advance(N_EARLY_SC[qtr])
                    if qtr < 3:
                        early_av_advance(N_EARLY_AV[qtr])

            # ---- attention stream + interleaved projection ---------------
            with (
                tc.tile_pool(name="pssc", bufs=3, space="PSUM") as pscp,
            ):
                pending_proj = []
                last_av = [None]

                def emit_proj(sb, gate, half):
                    # proj borrows a pssc-pool tile (its two 512-wide halves
                    # hold two m-chunks) so the scores pipeline stays deep.
                    s0 = sb * SBLK
                    pspr = pscp.tile([128, 2, SBLK], f32, name="pssc")
                    for mh in range(2):
                        m = half * 2 + mh
                        mm = nc.tensor.matmul(
                            pspr[:, mh, :], wp_sb[:, m, :],
                            outT2[:, s0:s0 + SBLK],
                            start=True, stop=True,
                        )
                        if gate is not None:
                            # Keep proj behind the attention stream so the
                            # norm chain (recip etc.) finishes off-PE first.
                            add_dep_helper(mm.ins, gate.ins, sync=False,
                                           reason="defer proj past boundary")
                        po = outpool.tile([128, SBLK], bf16, name="po")
                        # DMA can't read PSUM on this stack, so the drain
                        # must cross ACT or DVE. ACT in the steady state;
                        # alternate at the idle drain tail.
                        if gate is None and mh == 1:
                            nc.vector.tensor_copy(po[:], pspr[:, mh, :])
                        else:
                            nc.scalar.copy(po[:], pspr[:, mh, :])
                        nc.sync.dma_start(
                            out=o.ap()[m * 128:(m + 1) * 128, s0:s0 + SBLK],
                            in_=po[:],
                        )

                norm_q = []  # (due_step, phase_idx, state)
                step = [0]

                def run_due_norms():
                    while norm_q and norm_q[0][0] <= step[0]:
                        _, ph, st = norm_q.pop(0)
                        NORM_PHASES[ph](st)

                def av_pair(pu, pexpP):
                    # emit both heads' AV for the completed parity pair
                    for h in range(2):
                        last_av[0] = emit_av(pu, pexpP, h)
                        if pu[1] == TCH - 1:
                            sb = pu[0]
                            st = {"sb": sb, "h": h,
                                  "psav": psavs.pop((sb, h))}
                            # stagger the two heads' chains (they end at
                            # the same unit) and give the tiny norm DMAs
                            # headroom before their DVE consumers queue
                            norm_q.extend([(step[0] + 2 + h, 0, st),
                                           (step[0] + 6 + h, 1, st),
                                           (step[0] + 10 + h, 2, st)])
                            if h == 1:
                                pending_proj.append(sb)

                # AV consumption lags the scores/exp stream by AV_LAG units
                # so the in-order PE never stalls waiting for an exp that
                # just issued.
                start_i = exp_state["emitted"]
                pending = exp_state["pending"]
                # leftover prologue AV pairs may be half-consumed (head0
                # done): finish head1 first
                if exp_state.get("next_h", 0) == 1:
                    u0, p0 = pending.pop(0)
                    emit_av(u0, p0, 1)
                for i in range(start_i, len(units)):
                    u = units[i]
                    pssc = emit_scores(u, pscp)
                    if u[1] % 2 == 0:
                        curP = exppool.tile([128, 2, 2, SBLK], u8,
                                            name="expP")
                        exp_state["curP"] = curP
                    else:
                        curP = exp_state["curP"]
                    emit_exp(pssc[:], exp_dest(u, curP), is_dve_unit(i))
                    if u[1] % 2 == 1:
                        pending.append((u, curP))
                    step[0] += 1
                    run_due_norms()
                    if len(pending) > AV_LAG // 2:
                        av_pair(*pending.pop(0))
                    if pending_proj and (i % TCH) == 20:
                        emit_proj(pending_proj[0], last_av[0], 0)
                    elif pending_proj and (i % TCH) == 28:
                        emit_proj(pending_proj.pop(0), last_av[0], 1)
                for pu, pexpP in pending:
                    step[0] += 1
                    run_due_norms()
                    av_pair(pu, pexpP)
                step[0] += 99
                run_due_norms()
                for sb in pending_proj:
                    emit_proj(sb, None, 0)
                    emit_proj(sb, None, 1)
            ctx_psav.__exit__(None, None, None)

    nc.compile()
    return nc


def _prep_core_inputs(c, x, Wq, bq, Wk, bk, Wv, bv, Wp, bp):
    import ml_dtypes

    b = c // 4
    hs = 128 * (c % 4)
    bft = ml_dtypes.bfloat16

    def wslice_T(W):
        # W[hs:hs+128, :].T rearranged to [p, cc, d]
        return np.ascontiguousarray(
            W[hs:hs + 128, :].T.reshape(CC, 128, 128).transpose(1, 0, 2)
        ).astype(bft)

    wp_arr = np.ascontiguousarray(
        Wp[:, hs:hs + 128].reshape(CC, 128, 128).transpose(2, 0, 1)
    ).astype(bft)

    return {
        "xb": np.ascontiguousarray(x[b].reshape(C, S)).astype(bft),
        "wq": wslice_T(Wq),
        "wk": wslice_T(Wk),
        "wv": wslice_T(Wv),
        "wp": wp_arr,
        "bq": np.ascontiguousarray(bq[hs:hs + 128, None]).astype(np.float32),
    }


def _ensure_ntff_hook():
    # bass_utils unconditionally imports antenv.axon_hooks when tracing is
    # requested; the shipped antenv stub lacks it. Provide it (and register
    # the ctypes NTFF hook) so BASS_TRACE=1 works; silently skip otherwise.
    import types
    try:
        import antenv
        try:
            import antenv.axon_hooks  # noqa: F401
            return
        except ImportError:
            pass
        _hook = [None]
        mod = types.ModuleType("antenv.axon_hooks")
        mod.set_axon_ntff_profile_hook = lambda h: _hook.__setitem__(0, h)
        mod.get_axon_ntff_profile_hook = lambda: _hook[0]
        sys.modules["antenv.axon_hooks"] = mod
        antenv.axon_hooks = mod
        from trn_agent_boot.trn_boot import _ntff_profile_via_ctypes
        mod.set_axon_ntff_profile_hook(
            _ntff_profile_via_ctypes("/opt/axon/libaxon_pjrt.so")
        )
    except Exception:
        pass


def kernel(x, Wq, bq, Wk, bk, Wv, bv, Wp, bp):
    global LAST_EXEC_NS, LAST_RESULTS
    from concourse.bass_utils import run_bass_kernel_spmd

    x, Wq, bq, Wk, bk, Wv, bv, Wp, bp = (
        np.asarray(a, dtype=np.float32)
        for a in (x, Wq, bq, Wk, bk, Wv, bv, Wp, bp)
    )

    if "nc" not in _cached:
        _cached["nc"] = _build()
    nc = _cached["nc"]

    in_maps = [
        _prep_core_inputs(c, x, Wq, bq, Wk, bk, Wv, bv, Wp, bp)
        for c in range(NCORES)
    ]
    trace = bool(os.environ.get("BASS_TRACE"))
    if trace:
        _ensure_ntff_hook()
    res = run_bass_kernel_spmd(nc, in_maps, core_ids=list(range(NCORES)),
                               trace=trace)
    LAST_RESULTS = res
    LAST_EXEC_NS = res.exec_time_ns

    # The projection bias (bp) and V's bias routed through the projection
    # (bv @ Wp^T) are constant per output channel: added host-side during
    # the partial-sum gather.
    bias_total = (bv.astype(np.float64) @ Wp.T.astype(np.float64)
                  + bp.astype(np.float64)).astype(np.float32)
    out = np.zeros((B, C, S), dtype=np.float32)
    for c in range(NCORES):
        out[c // 4] += res.results[c]["o"]
    out += bias_total[None, :, None]
    return out.reshape(B, C, HH, WW)


# revision 21
# speedup vs baseline: 1.7087x; 1.7087x over previous
"""Multi-head self-attention (B=2, C=512, H=W=64, 8 heads) on 8 Trainium2 cores.

Sharding: data-parallel over B x head-parallel (2 heads/core). Core c handles
batch b = c//4 and heads {2*(c%4), 2*(c%4)+1} -- a contiguous 128-wide slice of
the 512-dim channel space.

Structure (v4 -- row-tiled concurrent score matmuls):
  - x[b] viewed as [C, S] is tok^T already (S = H*W = 4096 tokens); x and all
    weights are shipped as bf16.
  - q^T computed as [d2=128, S] bf16 with bias; k^T stored UNPADDED as
    kT_both[128, S]: rows 0-63 = head0 dims, 64-127 = head1 dims (k bias
    dropped: its score terms are query-only and cancel in softmax).
  - scores: per unit (sb, t) = (512-query strip, 128-key chunk), TWO
    CONCURRENT row-tiled K=64 matmuls (head0 in PE row-group 0-1, head1 in
    row-group 2-3, via base_partition 0/64) write pssc[128, 2(head), 512]
    in two PSUM banks. 2x the old zero-padded K=128 formulation: the PE
    streams 64+64 moving rows while draining 2x128 outputs into 2 banks.
  - exp: scores/8 ~ N(0, 0.33) so no max-subtraction. Split across engines:
      * ACT: exp activation (scale=1/8) writing float8e4.
      * DVE: Schraudolph-style magic exp -- uint8 = trunc(score*log2e +
        56.15) bitcast as float8e4 is exp(score/8) * 2^0.019 with ~4% rms
        error (the uniform factor cancels in softmax). One tensor_scalar op.
    exp output lands in expP[128, 2(head), 2(chunk parity), 512] so that a
    pair of units (t=2i, 2i+1) forms the [Ki, 2, N] moving operand per head.
  - attn.V: fp8e4 DoubleRow matmuls (two 128-key chunks per pass, K=256;
    the DR matmul streams its 1024 fp8 moving columns at 2 B/cycle/lane =
    216 ns, measured): stationary v8[128, 2, 96] = [v(64) | ones | zero pad]
    (ones column yields the softmax denominator in psav row 64), moving
    expP[:, h, :, :] fp8, out psav[96, 512] f32, one per (sb, head).
  - normalization: the [1,512] denominator row is DMA-spread to [128,4]
    so the DVE reciprocal runs wide, then DMA'd back, gpsimd-broadcast,
    and multiplied into outT2. Emitted in 3 phases several units apart so
    the in-order DVE never head-of-line blocks on an in-flight DMA; AV
    numerators are copied off PSUM in phase 1 so the psav bank frees early.
  - projection: input-column sharded; proj matmuls borrow score-pool PSUM
    tiles; PSUM->SBUF copies run on ACT; bias terms (bp and bv@Wp^T) are
    added on the host during the partial-sum gather.
  - pipeline: scores/exp stream runs AV_LAG units ahead of the AV consumer;
    the first 24 units are interleaved into the QKV prologue.

Error: fp8 quantization of exp/V ~0.7%, magic-exp ~0.3%, bf16 ~0.1% =>
measured 8.4e-3 against the 2e-2 gate (norm-relative, diluted by the
bias-dominated output norm).
"""

import os
import sys

sys.path.insert(0, "/opt/trn_rl_repo")

import numpy as np

NCORES = 8
B, C, HH, WW = 2, 512, 64, 64
S = HH * WW            # 4096 tokens
NH, D = 8, 64          # heads, head dim
DSL = 128              # per-core d-slice (2 heads)
CC = C // 128          # 4 contraction chunks
TCH = S // 128         # 32 key chunks
T2 = TCH // 2          # 16 key chunk-pairs
SBLK = 512             # queries per attention strip
NSB = S // SBLK        # 8

LOG2E = float(1.4426950408889634)
MAGIC_C = float(os.environ.get("MAGIC_C", "56.15"))
# t indices (0..31 within each sb group) whose exp runs on DVE via magic trick
DVE_T = frozenset(
    int(t) for t in os.environ.get(
        "DVE_T",
        "2,4,7,9,12,14,17,19,22,24,27,29,30",
    ).split(",") if t != ""
)
AV_LAG = int(os.environ.get("AV_LAG", "4"))


_cached = {}

LAST_EXEC_NS = None
LAST_RESULTS = None


def _build():
    import concourse.mybir as mybir
    import concourse.tile as tile
    from bass_rust import add_dep_helper
    from concourse import bacc
    from concourse.masks import make_identity

    f32 = mybir.dt.float32
    f32r = mybir.dt.float32r
    bf16 = mybir.dt.bfloat16
    f8 = mybir.dt.float8e4
    u8 = mybir.dt.uint8
    AF = mybir.ActivationFunctionType
    DR = mybir.MatmulPerfMode.DoubleRow

    nc = bacc.Bacc("TRN2", target_bir_lowering=False, debug=False,
                   num_devices=NCORES)

    xb = nc.dram_tensor("xb", [C, S], bf16, kind="ExternalInput")
    wq = nc.dram_tensor("wq", [128, CC, 128], bf16, kind="ExternalInput")
    wk = nc.dram_tensor("wk", [128, CC, 128], bf16, kind="ExternalInput")
    wv = nc.dram_tensor("wv", [128, CC, 128], bf16, kind="ExternalInput")
    wp = nc.dram_tensor("wp", [128, CC, 128], bf16, kind="ExternalInput")
    bq = nc.dram_tensor("bq", [128, 1], f32, kind="ExternalInput")
    o = nc.dram_tensor("o", [C, S], bf16, kind="ExternalOutput")

    with tile.TileContext(nc) as tc:
        with (
            tc.tile_pool(name="weights", bufs=1) as wpool,
            tc.tile_pool(name="tok", bufs=1) as tokpool,
            tc.tile_pool(name="qkv", bufs=1) as qkvpool,
            tc.tile_pool(name="exps", bufs=6) as exppool,
            tc.tile_pool(name="norm", bufs=10) as normpool,
            tc.tile_pool(name="outp", bufs=5) as outpool,
        ):
            wq_sb = wpool.tile([128, CC, 128], bf16, name="wq_sb")
            nc.sync.dma_start(out=wq_sb[:], in_=wq.ap())
            wk_sb = wpool.tile([128, CC, 128], bf16, name="wk_sb")
            nc.sync.dma_start(out=wk_sb[:], in_=wk.ap())
            wv_sb = wpool.tile([128, CC, 128], bf16, name="wv_sb")
            nc.sync.dma_start(out=wv_sb[:], in_=wv.ap())
            wp_sb = wpool.tile([128, CC, 128], bf16, name="wp_sb")
            nc.sync.dma_start(out=wp_sb[:], in_=wp.ap())
            bq_sb = wpool.tile([128, 1], f32, name="bq_sb")
            nc.sync.dma_start(out=bq_sb[:], in_=bq.ap())

            # tok^T in [partition, c_chunk, s] layout; DMA rearranges rows.
            tok_sb = tokpool.tile([128, CC, S], bf16, name="tok_sb")
            x_re = xb.ap().rearrange("(cc p) s -> p cc s", p=128)
            for qtr in range(4):
                for hf in range(2):
                    for cc in range(CC):
                        a = qtr * (S // 4) + hf * (S // 8)
                        sl = slice(a, a + S // 8)
                        nc.sync.dma_start(out=tok_sb[:, cc, sl],
                                          in_=x_re[:, cc, sl])

            # q/k in bf16. kT_both rows 0-63 = head0 dims, 64-127 = head1.
            qT2 = qkvpool.tile([128, S], bf16, name="qT2")
            kT_both = qkvpool.tile([128, S], bf16, name="kT_both")
            # v in fp8, chunk-pair layout for DoubleRow: [t, t2, j, 96] with
            # a fused ones column (65th) per chunk for the denominator.
            # DoubleRow stationary free (2*MV) must be a multiple of 64.
            MV = 96
            v8_0 = qkvpool.tile([128, T2, 2, MV], f8, name="v8_0")
            v8_1 = qkvpool.tile([128, T2, 2, MV], f8, name="v8_1")
            ones32 = qkvpool.tile([128, T2, 2], f32, name="ones32")
            nc.vector.memset(ones32[:], 1.0)
            nc.vector.tensor_copy(v8_0[:, :, :, 64], ones32[:])
            nc.vector.tensor_copy(v8_1[:, :, :, 64], ones32[:])
            zpad = qkvpool.tile([128, T2, 2, MV - 65], f32, name="zpad")
            nc.vector.memset(zpad[:], 0.0)
            nc.vector.tensor_copy(v8_0[:, :, :, 65:MV], zpad[:])
            nc.vector.tensor_copy(v8_1[:, :, :, 65:MV], zpad[:])

            outT2 = qkvpool.tile([128, S], bf16, name="outT2")

            ident = qkvpool.tile([128, 128], f32, name="ident")
            make_identity(nc, ident[:])

            ctx_psav = tc.tile_pool(name="psav", bufs=2, space="PSUM")
            pavp = ctx_psav.__enter__()
            psavs = {}
            exp_state = {"emitted": 0, "av_done": 0, "pending": []}

            # ---- unit schedule ------------------------------------------
            # A unit is (sb, t): one 512-query strip, one 128-key chunk,
            # both heads (concurrent row-tiled matmuls). AV consumes units
            # in chunk-parity pairs per head, accumulating over t2 = t//2
            # within each (sb, head) group. The first 24 units front-load
            # into the QKV prologue (chunk availability grows with token
            # quarters), at most 2 (sb, head) AV groups in flight.
            units = [(sb, t) for sb in range(NSB) for t in range(TCH)]
            N_EARLY_SC = [8, 16, 24]
            N_EARLY_AV = [6, 14, 22]   # AV emissions = (pair, head) steps

            def is_dve_unit(i):
                if i < 20:
                    return False  # prologue units: DVE is busy with QKV
                return units[i][1] in DVE_T

            def emit_exp(pssc_ap, out_ap, dve):
                # out_ap: [128, 2, 512] u8 view (head-major, parity slice)
                if dve:
                    nc.vector.tensor_scalar(
                        out_ap, pssc_ap, LOG2E, MAGIC_C,
                        mybir.AluOpType.mult, mybir.AluOpType.add,
                    )
                else:
                    nc.scalar.activation(out_ap.bitcast(f8), pssc_ap,
                                         AF.Exp, scale=0.125)

            def emit_scores(u, pool):
                sb, t = u
                s0 = sb * SBLK
                t0 = t * 128
                pssc = pool.tile([128, 2, SBLK], f32, name="pssc")
                for h in range(2):
                    nc.tensor.matmul(
                        pssc[:, h, :],
                        kT_both[h * 64:(h + 1) * 64, t0:t0 + 128],
                        qT2[h * 64:(h + 1) * 64, s0:s0 + SBLK],
                        start=True, stop=True,
                    )
                return pssc

            # expP per unit-pair: [128, 2(parity), 2(head), 512] -- parity
            # outer so each unit's exp writes one contiguous [128, 1024]
            # range (strided ACT output costs ~12%).
            def exp_dest(u, expP):
                return expP[:, u[1] % 2, :, :]

            def emit_av(u, expP, h):
                # u = the ODD unit of the pair; expP holds both parities
                sb, t = u
                t2 = t // 2
                v8 = v8_0 if h == 0 else v8_1
                if (sb, h) not in psavs:
                    psavs[(sb, h)] = pavp.tile([MV, SBLK], f32, name="psav")
                psav = psavs[(sb, h)]
                return nc.tensor.matmul(
                    psav[:],
                    v8[:, t2, :, :],
                    expP[:, :, h, :].bitcast(f8),
                    start=(t2 == 0), stop=(t2 == T2 - 1),
                    perf_mode=DR,
                )

            # The normalization chain (denominator reciprocal + broadcast +
            # multiply) crosses engines five times; spread the [1, 512]
            # denominator row across 128 partitions with a pair of tiny
            # SBUF<->SBUF DMAs and run the reciprocal at [128, 4]. The three
            # DVE steps are emitted a few units apart (phases) so the
            # in-order DVE never head-of-line blocks on an in-flight DMA.
            def norm_ph1(st):
                # one copy moves numerators AND the denominator row (65)
                # off PSUM; the spread-DMA then reads the SBUF copy.
                st["avs"] = normpool.tile([65, SBLK], f32, name="avs")
                nc.vector.tensor_copy(st["avs"][:], st["psav"][0:65, :])
                st["dsp"] = normpool.tile([128, SBLK // 128], f32, name="dsp")
                nc.sync.dma_start(out=st["dsp"][:], in_=st["avs"][64:65, :])

            def norm_ph2(st):
                st["rsp"] = normpool.tile([128, SBLK // 128], f32, name="rsp")
                nc.vector.reciprocal(st["rsp"][:], st["dsp"][:])
                st["recip"] = normpool.tile([1, SBLK], f32, name="recip")
                nc.sync.dma_start(out=st["recip"][:], in_=st["rsp"][:])
                st["rb"] = normpool.tile([64, SBLK], f32, name="rb")
                nc.gpsimd.partition_broadcast(st["rb"][:], st["recip"][:])

            def norm_ph3(st):
                sb, h = st["sb"], st["h"]
                p0 = h * 64
                nc.vector.tensor_mul(
                    outT2[p0:p0 + 64, sb * SBLK:(sb + 1) * SBLK],
                    st["avs"][0:64, :], st["rb"][:],
                )

            NORM_PHASES = (norm_ph1, norm_ph2, norm_ph3)

            # ---- fused Q/K/V prologue, quarter-major so compute chases
            # the x DMA. V is computed transposed (efficient N=512 matmuls)
            # and flipped into [t, d] fp8 layout with PE transposes.
            with (
                tc.tile_pool(name="psqk", bufs=2, space="PSUM") as pqkp,
                tc.tile_pool(name="pssce", bufs=2, space="PSUM") as pscep,
                tc.tile_pool(name="vt", bufs=2) as vtpool,
            ):
                def early_unit(i):
                    # scores + exp for unit i (sb=0, t=i) in prologue PSUM
                    u = units[i]
                    pssc = emit_scores(u, pscep)
                    if u[1] % 2 == 0:
                        expP = exppool.tile([128, 2, 2, SBLK], u8,
                                            name="expP")
                        exp_state["curP"] = expP
                    else:
                        expP = exp_state["curP"]
                    # prologue exp always on ACT (DVE busy with QKV), split
                    # per head so each [128, 512] lands as one op
                    nc.scalar.activation(
                        exp_dest(u, expP).bitcast(f8), pssc[:],
                        AF.Exp, scale=0.125)
                    return expP

                def early_scores_advance(k):
                    st = exp_state
                    while st["emitted"] < k:
                        i = st["emitted"]
                        expP = early_unit(i)
                        if units[i][1] % 2 == 1:
                            st["pending"].append((units[i], expP))
                        st["emitted"] = i + 1

                def early_av_advance(k):
                    # k counts AV emissions (pair x head steps)
                    st = exp_state
                    while st["av_done"] < k and st["pending"]:
                        u, expP = st["pending"][0]
                        h = st.get("next_h", 0)
                        emit_av(u, expP, h)
                        st["av_done"] += 1
                        if h == 0:
                            st["next_h"] = 1
                        else:
                            st["next_h"] = 0
                            st["pending"].pop(0)

                for qtr in range(4):
                    for which in range(3):
                        w_sb = (wq_sb, wk_sb, wv_sb)[which]
                        for scq in range(2):
                            sc = qtr * 2 + scq
                            s0 = sc * 512
                            psqk = pqkp.tile([128, 512], f32, name="psqk")
                            for cc in range(CC):
                                nc.tensor.matmul(
                                    psqk[:],
                                    w_sb[:, cc, :],
                                    tok_sb[:, cc, s0:s0 + 512],
                                    start=(cc == 0), stop=(cc == CC - 1),
                                )
                            if which == 0:
                                nc.vector.tensor_scalar_add(
                                    qT2[:, s0:s0 + 512], psqk[:], bq_sb[:, 0:1]
                                )
                            elif which == 1:
                                # no k bias: its score terms are query-only
                                # and cancel in softmax
                                nc.vector.tensor_copy(
                                    kT_both[:, s0:s0 + 512], psqk[:]
                                )
                            else:
                                vt = vtpool.tile([128, 512], f32r, name="vt")
                                nc.vector.tensor_copy(vt[:], psqk[:])
                                # 4 transposes into one PSUM tile, then two
                                # strided fp8 copies peel the head halves.
                                pst4 = pqkp.tile([128, 512], f32, name="psqk")
                                for tt in range(4):
                                    nc.tensor.transpose(
                                        pst4[:, tt * 128:(tt + 1) * 128],
                                        vt[:, tt * 128:(tt + 1) * 128]
                                        .bitcast(f32),
                                        ident[:],
                                    )
                                # chunks sc*4 .. sc*4+3 -> t2 = sc*2, sc*2+1
                                t2a = sc * 2
                                src0 = pst4[:].rearrange(
                                    "p (c d) -> p c d", c=4)[:, :, 0:64]
                                src1 = pst4[:].rearrange(
                                    "p (c d) -> p c d", c=4)[:, :, 64:128]
                                dst0 = v8_0[:, t2a:t2a + 2, :, 0:64]
                                dst1 = v8_1[:, t2a:t2a + 2, :, 0:64]
                                nc.vector.tensor_copy(dst0, src0)
                                nc.vector.tensor_copy(dst1, src1)
                        if qtr < 3 and which == 1:
                            early_scores_advance(N_EARLY_SC[qtr])
                    if qtr < 3:
                        early_av_advance(N_EARLY_AV[qtr])

            # ---- attention stream + interleaved projection ---------------
            with (
                tc.tile_pool(name="pssc", bufs=3, space="PSUM") as pscp,
            ):
                pending_proj = []
                last_av = [None]

                def emit_proj(sb, gate, half):
                    # proj borrows a pssc-pool tile (its two 512-wide halves
                    # hold two m-chunks) so the scores pipeline stays deep.
                    s0 = sb * SBLK
                    pspr = pscp.tile([128, 2, SBLK], f32, name="pssc")
                    for mh in range(2):
                        m = half * 2 + mh
                        mm = nc.tensor.matmul(
                            pspr[:, mh, :], wp_sb[:, m, :],
                            outT2[:, s0:s0 + SBLK],
                            start=True, stop=True,
                        )
                        if gate is not None:
                            # Keep proj behind the attention stream so the
                            # norm chain (recip etc.) finishes off-PE first.
                            add_dep_helper(mm.ins, gate.ins, sync=False,
                                           reason="defer proj past boundary")
                    # DMA can't read PSUM on this stack, so the drain must
                    # cross ACT or DVE; one wide 1024-FD copy per half
                    # amortizes the per-op overhead. DVE at the idle tail.
                    po = outpool.tile([128, 2, SBLK], bf16, name="po")
                    if gate is None and half == 1:
                        nc.vector.tensor_copy(po[:], pspr[:])
                    else:
                        nc.scalar.copy(po[:], pspr[:])
                    for mh in range(2):
                        m = half * 2 + mh
                        nc.sync.dma_start(
                            out=o.ap()[m * 128:(m + 1) * 128, s0:s0 + SBLK],
                            in_=po[:, mh, :],
                        )

                norm_q = []  # (due_step, phase_idx, state)
                step = [0]

                def run_due_norms():
                    while norm_q and norm_q[0][0] <= step[0]:
                        _, ph, st = norm_q.pop(0)
                        NORM_PHASES[ph](st)

                def av_pair(pu, pexpP):
                    # emit both heads' AV for the completed parity pair
                    for h in range(2):
                        last_av[0] = emit_av(pu, pexpP, h)
                        if pu[1] == TCH - 1:
                            sb = pu[0]
                            st = {"sb": sb, "h": h,
                                  "psav": psavs.pop((sb, h))}
                            # stagger the two heads' chains (they end at
                            # the same unit) and give the tiny norm DMAs
                            # headroom before their DVE consumers queue
                            norm_q.extend([(step[0] + 2 + h, 0, st),
                                           (step[0] + 6 + h, 1, st),
                                           (step[0] + 10 + h, 2, st)])
                            if h == 1:
                                pending_proj.append(sb)

                # AV consumption lags the scores/exp stream by AV_LAG units
                # so the in-order PE never stalls waiting for an exp that
                # just issued.
                start_i = exp_state["emitted"]
                pending = exp_state["pending"]
                # leftover prologue AV pairs may be half-consumed (head0
                # done): finish head1 first
                if exp_state.get("next_h", 0) == 1:
                    u0, p0 = pending.pop(0)
                    emit_av(u0, p0, 1)
                for i in range(start_i, len(units)):
                    u = units[i]
                    pssc = emit_scores(u, pscp)
                    if u[1] % 2 == 0:
                        curP = exppool.tile([128, 2, 2, SBLK], u8,
                                            name="expP")
                        exp_state["curP"] = curP
                    else:
                        curP = exp_state["curP"]
                    emit_exp(pssc[:], exp_dest(u, curP), is_dve_unit(i))
                    if u[1] % 2 == 1:
                        pending.append((u, curP))
                    step[0] += 1
                    run_due_norms()
                    if len(pending) > AV_LAG // 2:
                        av_pair(*pending.pop(0))
                    if pending_proj and (i % TCH) == 20:
                        emit_proj(pending_proj[0], last_av[0], 0)
                    elif pending_proj and (i % TCH) == 28:
                        emit_proj(pending_proj.pop(0), last_av[0], 1)
                for pu, pexpP in pending:
                    step[0] += 1
                    run_due_norms()
                    av_pair(pu, pexpP)
                step[0] += 99
                run_due_norms()
                for sb in pending_proj:
                    emit_proj(sb, None, 0)
                    emit_proj(sb, None, 1)
            ctx_psav.__exit__(None, None, None)

    nc.compile()
    return nc


def _prep_core_inputs(c, x, Wq, bq, Wk, bk, Wv, bv, Wp, bp):
    import ml_dtypes

    b = c // 4
    hs = 128 * (c % 4)
    bft = ml_dtypes.bfloat16

    def wslice_T(W):
        # W[hs:hs+128, :].T rearranged to [p, cc, d]
        return np.ascontiguousarray(
            W[hs:hs + 128, :].T.reshape(CC, 128, 128).transpose(1, 0, 2)
        ).astype(bft)

    wp_arr = np.ascontiguousarray(
        Wp[:, hs:hs + 128].reshape(CC, 128, 128).transpose(2, 0, 1)
    ).astype(bft)

    return {
        "xb": np.ascontiguousarray(x[b].reshape(C, S)).astype(bft),
        "wq": wslice_T(Wq),
        "wk": wslice_T(Wk),
        "wv": wslice_T(Wv),
        "wp": wp_arr,
        "bq": np.ascontiguousarray(bq[hs:hs + 128, None]).astype(np.float32),
    }


def _ensure_ntff_hook():
    # bass_utils unconditionally imports antenv.axon_hooks when tracing is
    # requested; the shipped antenv stub lacks it. Provide it (and register
    # the ctypes NTFF hook) so BASS_TRACE=1 works; silently skip otherwise.
    import types
    try:
        import antenv
        try:
            import antenv.axon_hooks  # noqa: F401
            return
        except ImportError:
            pass
        _hook = [None]
        mod = types.ModuleType("antenv.axon_hooks")
        mod.set_axon_ntff_profile_hook = lambda h: _hook.__setitem__(0, h)
        mod.get_axon_ntff_profile_hook = lambda: _hook[0]
        sys.modules["antenv.axon_hooks"] = mod
        antenv.axon_hooks = mod
        from trn_agent_boot.trn_boot import _ntff_profile_via_ctypes
        mod.set_axon_ntff_profile_hook(
            _ntff_profile_via_ctypes("/opt/axon/libaxon_pjrt.so")
        )
    except Exception:
        pass


def kernel(x, Wq, bq, Wk, bk, Wv, bv, Wp, bp):
    global LAST_EXEC_NS, LAST_RESULTS
    from concourse.bass_utils import run_bass_kernel_spmd

    x, Wq, bq, Wk, bk, Wv, bv, Wp, bp = (
        np.asarray(a, dtype=np.float32)
        for a in (x, Wq, bq, Wk, bk, Wv, bv, Wp, bp)
    )

    if "nc" not in _cached:
        _cached["nc"] = _build()
    nc = _cached["nc"]

    in_maps = [
        _prep_core_inputs(c, x, Wq, bq, Wk, bk, Wv, bv, Wp, bp)
        for c in range(NCORES)
    ]
    trace = bool(os.environ.get("BASS_TRACE"))
    if trace:
        _ensure_ntff_hook()
    res = run_bass_kernel_spmd(nc, in_maps, core_ids=list(range(NCORES)),
                               trace=trace)
    LAST_RESULTS = res
    LAST_EXEC_NS = res.exec_time_ns

    # The projection bias (bp) and V's bias routed through the projection
    # (bv @ Wp^T) are constant per output channel: added host-side during
    # the partial-sum gather.
    bias_total = (bv.astype(np.float64) @ Wp.T.astype(np.float64)
                  + bp.astype(np.float64)).astype(np.float32)
    out = np.zeros((B, C, S), dtype=np.float32)
    for c in range(NCORES):
        out[c // 4] += res.results[c]["o"]
    out += bias_total[None, :, None]
    return out.reshape(B, C, HH, WW)
